# revision 21
# baseline (speedup 1.0000x reference)
"""CapsuleLayer routing kernel for 8 Trainium2 NeuronCores.

Problem (full shapes): x [B=32, N=2048, IC=16] fp32,
route_weights [N=2048, K=32, IC=16, OC=32] fp32.
  priors = einsum('bni,nkio->bnko', x, W)
  3 routing iterations (softmax over K, weighted sum over N, squash)
  output = squash(s2) shaped [B, 1, K, 1, OC].

Sharding: N (nodes) is sharded 8 ways (256 nodes/core); the
route_weights shard (bf16, 8.4MB) stays SBUF-resident; priors are
recomputed on the PE each routing pass and never touch HBM.  The only
cross-core traffic is an AllReduce of s [B, K*OC] (128KB fp32) per
iteration.

Per-core per-pass structure (32 W-tiles of 8 nodes = 2 groups of 4):
  PE   : priors P[(4n,32b),(k,o)] via block-diag-x stationary matmuls
  ACT  : P PSUM->SBUF bf16 copy; fused exp(L)+o-expand with accumulated Z
  DVE  : t = P*v_rep; d = tree-reduce_o(t); zblk = ones*(1/Z);
         wp = exp_expanded * P
  PE   : s += zblk.T @ wp  (sums the 4 nodes and applies the softmax
         1/Z; PSUM-accumulates across all groups)
"""

import numpy as np
import ml_dtypes

B, NLOC, K, IC, OC = 32, 256, 32, 16, 32
NCORES = 8
N = NLOC * NCORES
KO = K * OC            # 1024
NT = NLOC // 8         # 32 W-tiles of 8 nodes
NGRP = NLOC // 4       # 64 groups of 4 nodes

_CACHE = {}


def _build_bass():
    import concourse.bass as bass
    import concourse.mybir as mybir
    from concourse import bacc, tile

    dt = mybir.dt
    AF = mybir.ActivationFunctionType
    ALU = mybir.AluOpType

    nc = bacc.Bacc("TRN2", target_bir_lowering=False)

    wmov_d = nc.declare_dram_parameter("wmov", [NT, 128, KO], dt.bfloat16, isOutput=False)
    xblk_d = nc.declare_dram_parameter("xblk", [NT, 128, 128], dt.bfloat16, isOutput=False)
    xall_d = nc.declare_dram_parameter("xall", [NT, 128, B], dt.bfloat16, isOutput=False)
    ones_d = nc.declare_dram_parameter("onesblk", [128, B], dt.bfloat16, isOutput=False)
    vout_d = nc.declare_dram_parameter("vout", [B, KO], dt.float32, isOutput=True)

    groups = [list(range(NCORES))]

    with tile.TileContext(nc) as tc:
        with (
            tc.tile_pool(name="wsb", bufs=1) as wpool,
            tc.tile_pool(name="persist", bufs=1) as ppool,
            tc.tile_pool(name="ltiles", bufs=NT) as lpool,
            tc.tile_pool(name="psb", bufs=6) as psb_pool,
            tc.tile_pool(name="tsb", bufs=3) as t_pool,
            tc.tile_pool(name="tree", bufs=2) as u_pool,
            tc.tile_pool(name="wp", bufs=3) as wp_pool,
            tc.tile_pool(name="eexp", bufs=3) as e_pool,
            tc.tile_pool(name="sm", bufs=6) as sm_pool,
            tc.tile_pool(name="vv", bufs=1) as v_pool,
            tc.tile_pool(name="ppsum", bufs=3, space="PSUM") as ppsum_pool,
            tc.tile_pool(name="spsum", bufs=1, space="PSUM") as spsum_pool,
            tc.tile_pool(name="dram", bufs=4, space="DRAM") as dram_pool,
        ):
            wsb = wpool.tile([128, NT * KO], dt.bfloat16, tag="wsb")
            xblk = ppool.tile([128, NT * 128], dt.bfloat16, tag="xblk")
            xall = ppool.tile([128, NT * B], dt.bfloat16, tag="xall")
            onesblk = ppool.tile([128, B], dt.bfloat16, tag="ones")

            # CC warmup: a tiny AllReduce issued first, overlapping the
            # input loads, absorbs the collective stack's cold start.
            warm_in = dram_pool.tile([1, 4], dt.float32, tag="warmin")
            warm_out = dram_pool.tile([1, 4], dt.float32, tag="warmout")
            warm_sb = v_pool.tile([1, 4], dt.float32, tag="warmsb")
            nc.vector.memset(warm_sb[:], 0.0)
            nc.gpsimd.dma_start(out=warm_in[:], in_=warm_sb[:])
            nc.gpsimd.collective_compute(
                "AllReduce", ALU.add, replica_groups=groups,
                ins=[warm_in.opt()], outs=[warm_out.opt()],
            )

            # batched input loads; issue spread over 4 engine queues so
            # descriptor generation isn't serialized on one sequencer
            qs = [nc.sync, nc.scalar, nc.sync, nc.scalar]
            for c in range(16):
                qs[c % 4].dma_start(
                    out=wsb[:].rearrange("p (t f) -> p t f", t=NT)[:, 2 * c:2 * c + 2, :],
                    in_=wmov_d[:].transpose([1, 0, 2])[:, 2 * c:2 * c + 2, :],
                )
            for c in range(2):
                qs[c].dma_start(
                    out=xblk[:].rearrange("p (t f) -> p t f", t=NT)[:, 16 * c:16 * c + 16, :],
                    in_=xblk_d[:].transpose([1, 0, 2])[:, 16 * c:16 * c + 16, :],
                )
            qs[2].dma_start(
                out=xall[:].rearrange("p (t f) -> p t f", t=NT),
                in_=xall_d[:].transpose([1, 0, 2]),
            )
            qs[3].dma_start(out=onesblk[:], in_=ones_d[:])

            # persistent logits tiles, one [128(4n,32b), 2*K] per W-tile
            ltiles = [lpool.tile([128, 2 * K], dt.float32, tag="L", name=f"L{t}")
                      for t in range(NT)]

            def allreduce_squash(s_ps0, s_ps1, last, it):
                """PSUM s halves -> AllReduce -> squash -> fresh vrep tile."""
                sfull = v_pool.tile([B, KO], dt.float32, tag="sfull",
                                    name=f"sfull{it}", bufs=2)
                vf32 = v_pool.tile([B, KO], dt.float32, tag="vf32",
                                   name=f"vf32{it}", bufs=2)
                sq1 = v_pool.tile([B, KO], dt.float32, tag="sq1",
                                  name=f"sq1{it}", bufs=2)
                nrm = v_pool.tile([B, K], dt.float32, tag="nrm",
                                  name=f"nrm{it}", bufs=3)
                nrm1 = v_pool.tile([B, K], dt.float32, tag="nrm1",
                                   name=f"nrm1{it}", bufs=3)
                den = v_pool.tile([B, K], dt.float32, tag="den",
                                  name=f"den{it}", bufs=3)
                rden = v_pool.tile([B, K], dt.float32, tag="rden",
                                   name=f"rden{it}", bufs=3)
                scal = v_pool.tile([B, K], dt.float32, tag="scal",
                                   name=f"scal{it}", bufs=3)
                nc.scalar.copy(out=sfull[:, 0:512], in_=s_ps0[:])
                nc.scalar.copy(out=sfull[:, 512:1024], in_=s_ps1[:])
                cc_in = dram_pool.tile([B, KO], dt.float32, tag="ccin")
                cc_out = dram_pool.tile([B, KO], dt.float32, tag="ccout")
                nc.gpsimd.dma_start(out=cc_in[:], in_=sfull[:])
                nc.gpsimd.collective_compute(
                    "AllReduce", ALU.add, replica_groups=groups,
                    ins=[cc_in.opt()], outs=[cc_out.opt()],
                )
                sred = v_pool.tile([B, KO], dt.float32, tag="sred",
                                   name=f"sred{it}", bufs=2)
                nc.gpsimd.dma_start(out=sred[:], in_=cc_out[:])
                # squash: v = s * nrm/((1+nrm)*sqrt(nrm)), nrm = sum_o s^2
                nc.scalar.activation(out=sq1[:], in_=sred[:], func=AF.Square)
                nc.vector.reduce_sum(
                    out=nrm[:], in_=sq1[:].rearrange("p (k o) -> p k o", k=K),
                    axis=mybir.AxisListType.X,
                )
                nc.vector.tensor_scalar_add(nrm1[:], nrm[:], 1.0)
                nc.scalar.activation(out=den[:], in_=nrm[:], func=AF.Sqrt)
                nc.vector.tensor_mul(den[:], den[:], nrm1[:])
                nc.vector.reciprocal(rden[:], den[:])
                nc.vector.tensor_mul(scal[:], nrm[:], rden[:])
                nc.vector.tensor_mul(
                    vf32[:].rearrange("p (k o) -> p k o", k=K),
                    sred[:].rearrange("p (k o) -> p k o", k=K),
                    scal[:].unsqueeze(2).broadcast_to((B, K, OC)),
                )
                if last:
                    nc.gpsimd.dma_start(out=vout_d[:], in_=vf32[:])
                    return None
                vbf = v_pool.tile([B, KO], dt.bfloat16, tag="vbf",
                                  name=f"vbf{it}", bufs=2)
                vrep = v_pool.tile([128, KO], dt.bfloat16, tag="vrep",
                                   name=f"vrep{it}", bufs=2)
                nc.vector.tensor_copy(vbf[:], vf32[:])
                for r in range(4):
                    nc.gpsimd.dma_start(
                        out=vrep[r * 32:(r + 1) * 32, :], in_=vbf[:]
                    )
                return vrep

            # ---------- pass A: s0 = (1/K) sum_n priors (direct matmul) -----
            s0a = spsum_pool.tile([B, 512], dt.float32, tag="sacc0")
            s0b = spsum_pool.tile([B, 512], dt.float32, tag="sacc1")
            for t in range(NT):
                nc.tensor.matmul(
                    out=s0a[:], lhsT=xall[:, t * B:(t + 1) * B],
                    rhs=wsb[:, t * KO:t * KO + 512],
                    start=(t == 0), stop=(t == NT - 1),
                )
                nc.tensor.matmul(
                    out=s0b[:], lhsT=xall[:, t * B:(t + 1) * B],
                    rhs=wsb[:, t * KO + 512:(t + 1) * KO],
                    start=(t == 0), stop=(t == NT - 1),
                )
            vrep = allreduce_squash(s0a, s0b, last=False, it=0)

            # ---------- passes B (iter1) and C (iter2) ----------------------
            for it in (1, 2):
                sa = spsum_pool.tile([B, 512], dt.float32, tag="sacc0")
                sb = spsum_pool.tile([B, 512], dt.float32, tag="sacc1")
                for t in range(NT):
                    pp = [ppsum_pool.tile([128, KO], dt.float32, tag="pp",
                                          name=f"pp{it}_{t}_{s}")
                          for s in (0, 1)]
                    for s in (0, 1):
                        lhs = xblk[s * 64:(s + 1) * 64, t * 128:(t + 1) * 128]
                        for h in (0, 1):
                            nc.tensor.matmul(
                                out=pp[s][:, h * 512:(h + 1) * 512], lhsT=lhs,
                                rhs=wsb[s * 64:(s + 1) * 64,
                                        t * KO + h * 512:t * KO + (h + 1) * 512],
                                start=True, stop=True, skip_group_check=True,
                            )
                    psb = psb_pool.tile([128, 2 * KO], dt.bfloat16, tag="psb",
                                        name=f"psb{it}_{t}")
                    # balance ACT vs DVE: every other tile, the strip-1
                    # PSUM->SBUF copy goes to the vector engine instead
                    nc.scalar.copy(out=psb[:, 0:KO], in_=pp[0][:])
                    if t % 2 == 0:
                        nc.vector.tensor_copy(psb[:, KO:2 * KO], pp[1][:])
                    else:
                        nc.scalar.copy(out=psb[:, KO:2 * KO], in_=pp[1][:])
                    # t = P * v_rep over both groups at once
                    tt = t_pool.tile([128, 2 * KO], dt.bfloat16, tag="t",
                                     name=f"t{it}_{t}")
                    nc.vector.tensor_mul(
                        tt[:].rearrange("p (g f) -> p g f", g=2),
                        psb[:].rearrange("p (g f) -> p g f", g=2),
                        vrep[:].unsqueeze(1).broadcast_to((128, 2, KO)),
                    )
                    # tree reduce over o (bf16 2x adds, last level strided)
                    t3 = tt[:].rearrange("p (gk o) -> p gk o", o=32)
                    u1 = u_pool.tile([128, 64 * 16], dt.bfloat16, tag="u1",
                                     name=f"u1_{it}_{t}")
                    u1v = u1[:].rearrange("p (gk o) -> p gk o", o=16)
                    nc.vector.tensor_add(u1v, t3[:, :, 0:16], t3[:, :, 16:32])
                    u2 = u_pool.tile([128, 64 * 8], dt.bfloat16, tag="u2",
                                     name=f"u2_{it}_{t}")
                    u2v = u2[:].rearrange("p (gk o) -> p gk o", o=8)
                    nc.vector.tensor_add(u2v, u1v[:, :, 0:8], u1v[:, :, 8:16])
                    u3 = u_pool.tile([128, 64 * 4], dt.bfloat16, tag="u3",
                                     name=f"u3_{it}_{t}")
                    u3v = u3[:].rearrange("p (gk o) -> p gk o", o=4)
                    nc.vector.tensor_add(u3v, u2v[:, :, 0:4], u2v[:, :, 4:8])
                    u4 = u_pool.tile([128, 64 * 2], dt.bfloat16, tag="u4",
                                     name=f"u4_{it}_{t}")
                    u4v = u4[:].rearrange("p (gk o) -> p gk o", o=2)
                    nc.vector.tensor_add(u4v, u3v[:, :, 0:2], u3v[:, :, 2:4])
                    if it == 1:
                        nc.vector.tensor_add(
                            ltiles[t][:].unsqueeze(2),
                            u4v[:, :, 0:1], u4v[:, :, 1:2])
                    else:
                        dtmp = sm_pool.tile([128, 2 * K], dt.float32, tag="dtmp",
                                            name=f"dtmp{it}_{t}")
                        nc.vector.tensor_add(
                            dtmp[:].unsqueeze(2),
                            u4v[:, :, 0:1], u4v[:, :, 1:2])
                        nc.vector.tensor_add(ltiles[t][:], ltiles[t][:], dtmp[:])
                    # fused exp + o-expand per group, with Z accumulator
                    eexp = e_pool.tile([128, 2 * KO], dt.bfloat16, tag="eexp",
                                       name=f"eexp{it}_{t}")
                    zacc = sm_pool.tile([128, 2], dt.float32, tag="zacc",
                                        name=f"zacc{it}_{t}")
                    for g in (0, 1):
                        nc.scalar.activation(
                            out=eexp[:, g * KO:(g + 1) * KO].rearrange(
                                "p (k o) -> p k o", k=K),
                            in_=ltiles[t][:, g * K:(g + 1) * K]
                                .unsqueeze(2).broadcast_to((128, K, OC)),
                            func=AF.Exp,
                            accum_out=zacc[:, g:g + 1],
                        )
                    zr = sm_pool.tile([128, 2], dt.float32, tag="zr",
                                      name=f"zr{it}_{t}")
                    nc.vector.reciprocal(zr[:], zacc[:])
                    # wp = eexp * P  (unnormalized); 1/Z folded into zblk
                    wp = wp_pool.tile([128, 2 * KO], dt.bfloat16, tag="wp",
                                      name=f"wp{it}_{t}")
                    nc.vector.tensor_mul(wp[:], eexp[:], psb[:])
                    for g in (0, 1):
                        zblk = sm_pool.tile([128, B], dt.bfloat16, tag="zblk",
                                            name=f"zblk{it}_{t}_{g}")
                        # onesblk carries OC(=32)x so zblk = 32*delta/Zacc
                        nc.vector.tensor_scalar_mul(
                            zblk[:], onesblk[:], zr[:, g:g + 1])
                        gg = 2 * t + g
                        nc.tensor.matmul(
                            out=sa[:], lhsT=zblk[:],
                            rhs=wp[:, g * KO:g * KO + 512],
                            start=(gg == 0), stop=(gg == NGRP - 1),
                            skip_group_check=True,
                        )
                        nc.tensor.matmul(
                            out=sb[:], lhsT=zblk[:],
                            rhs=wp[:, g * KO + 512:(g + 1) * KO],
                            start=(gg == 0), stop=(gg == NGRP - 1),
                            skip_group_check=True,
                        )
                vrep = allreduce_squash(sa, sb, last=(it == 2), it=it)

    nc.compile()
    return nc


def _prep_inputs(x, route_weights):
    """Host-side shard + layout prep. Returns per-core in_maps."""
    bf16 = ml_dtypes.bfloat16
    xw = x.astype(np.float32)
    W = route_weights.astype(np.float32)
    in_maps = []
    for c in range(NCORES):
        n0 = c * NLOC
        xc = xw[:, n0:n0 + NLOC, :]          # [B, 256, IC]
        Wc = W[n0:n0 + NLOC]                 # [256, K, IC, OC]
        # wmov[t][s*64 + j*16 + i, k*OC + o] = W[8t+4s+j, k, i, o]
        wm = Wc.reshape(NT, 8, K, IC, OC)          # [t, node, k, i, o]
        wm = wm.transpose(0, 1, 3, 2, 4)           # [t, node, i, k, o]
        wmov = np.ascontiguousarray(
            wm.reshape(NT, 128, KO)).astype(bf16)
        # xblk[t][s*64 + j*16 + i, j'*32 + b] = x[b, 8t+4s+j, i] * (j==j')
        xb = np.zeros((NT, 2, 4, IC, 4, B), np.float32)
        xg = xc.transpose(1, 2, 0).reshape(NT, 2, 4, IC, B)  # [t,s,j,i,b]
        for j in range(4):
            xb[:, :, j, :, j, :] = xg[:, :, j, :, :]
        xblk = np.ascontiguousarray(
            xb.reshape(NT, 128, 128)).astype(bf16)
        # xall[t][(s*4+j)*16 + i, b] = x[b, n, i] / K
        xall = np.ascontiguousarray(
            (xg / K).reshape(NT, 128, B)).astype(bf16)
        # ones: delta(b,b') scaled by OC=32 to cancel the o-expansion in
        # the exp accumulator (Zacc = 32 * sum_k exp)
        ones = np.zeros((128, B), np.float32)
        for j in range(4):
            ones[j * 32 + np.arange(32), np.arange(32)] = float(OC)
        onesblk = ones.astype(bf16)
        in_maps.append({
            "wmov": wmov, "xblk": xblk, "xall": xall, "onesblk": onesblk,
        })
    return in_maps


def _get_nc():
    if "nc" not in _CACHE:
        _CACHE["nc"] = _build_bass()
    return _CACHE["nc"]


def kernel(x, route_weights, _trace=False, _trace_kwargs=None):
    from concourse.bass_utils import run_bass_kernel_spmd

    nc = _get_nc()
    in_maps = _prep_inputs(np.asarray(x), np.asarray(route_weights))
    res = run_bass_kernel_spmd(
        nc, in_maps, core_ids=list(range(NCORES)),
        trace=_trace, **(_trace_kwargs or {}),
    )
    out = res.results[0]["vout"].astype(np.float32)       # [B, K*OC]
    full = out.reshape(B, 1, K, 1, OC)
    if _trace:
        return full, res
    return full


# revision 22
# speedup vs baseline: 1.0307x; 1.0307x over previous
"""CapsuleLayer routing kernel for 8 Trainium2 NeuronCores.

Problem (full shapes): x [B=32, N=2048, IC=16] fp32,
route_weights [N=2048, K=32, IC=16, OC=32] fp32.
  priors = einsum('bni,nkio->bnko', x, W)
  3 routing iterations (softmax over K, weighted sum over N, squash)
  output = squash(s2) shaped [B, 1, K, 1, OC].

Sharding: N (nodes) is sharded 8 ways (256 nodes/core); the
route_weights shard (bf16, 8.4MB) stays SBUF-resident; priors are
recomputed on the PE each routing pass and never touch HBM.  The only
cross-core traffic is an AllReduce of s [B, K*OC] (128KB fp32) per
iteration.

Per-core per-pass structure (32 W-tiles of 8 nodes = 2 groups of 4):
  PE   : priors P[(4n,32b),(k,o)] via block-diag-x stationary matmuls
  ACT  : P PSUM->SBUF bf16 copy; fused exp(L)+o-expand with accumulated Z
  DVE  : t = P*v_rep; d = tree-reduce_o(t); zblk = ones*(1/Z);
         wp = exp_expanded * P
  PE   : s += zblk.T @ wp  (sums the 4 nodes and applies the softmax
         1/Z; PSUM-accumulates across all groups)
"""

import numpy as np
import ml_dtypes

B, NLOC, K, IC, OC = 32, 256, 32, 16, 32
NCORES = 8
N = NLOC * NCORES
KO = K * OC            # 1024
NT = NLOC // 8         # 32 W-tiles of 8 nodes
NGRP = NLOC // 4       # 64 groups of 4 nodes

_CACHE = {}


def _build_bass():
    import concourse.bass as bass
    import concourse.mybir as mybir
    from concourse import bacc, tile

    dt = mybir.dt
    AF = mybir.ActivationFunctionType
    ALU = mybir.AluOpType

    nc = bacc.Bacc("TRN2", target_bir_lowering=False)

    wmov_d = nc.declare_dram_parameter("wmov", [NT, 128, KO], dt.bfloat16, isOutput=False)
    xblk_d = nc.declare_dram_parameter("xblk", [NT, 128, 128], dt.bfloat16, isOutput=False)
    xall_d = nc.declare_dram_parameter("xall", [NT, 128, B], dt.bfloat16, isOutput=False)
    ones_d = nc.declare_dram_parameter("onesblk", [128, B], dt.bfloat16, isOutput=False)
    vout_d = nc.declare_dram_parameter("vout", [B, KO], dt.float32, isOutput=True)

    groups = [list(range(NCORES))]

    with tile.TileContext(nc) as tc:
        with (
            tc.tile_pool(name="wsb", bufs=1) as wpool,
            tc.tile_pool(name="persist", bufs=1) as ppool,
            tc.tile_pool(name="ltiles", bufs=NT) as lpool,
            tc.tile_pool(name="psb", bufs=6) as psb_pool,
            tc.tile_pool(name="tsb", bufs=3) as t_pool,
            tc.tile_pool(name="tree", bufs=2) as u_pool,
            tc.tile_pool(name="wp", bufs=3) as wp_pool,
            tc.tile_pool(name="eexp", bufs=3) as e_pool,
            tc.tile_pool(name="sm", bufs=6) as sm_pool,
            tc.tile_pool(name="vv", bufs=1) as v_pool,
            tc.tile_pool(name="ppsum", bufs=3, space="PSUM") as ppsum_pool,
            tc.tile_pool(name="spsum", bufs=1, space="PSUM") as spsum_pool,
            tc.tile_pool(name="dram", bufs=4, space="DRAM") as dram_pool,
        ):
            wsb = wpool.tile([128, NT * KO], dt.bfloat16, tag="wsb")
            xblk = ppool.tile([128, NT * 128], dt.bfloat16, tag="xblk")
            xall = ppool.tile([128, NT * B], dt.bfloat16, tag="xall")
            onesblk = ppool.tile([128, B], dt.bfloat16, tag="ones")

            # CC warmup: a tiny AllReduce issued first, overlapping the
            # input loads, absorbs the collective stack's cold start.
            warm_in = dram_pool.tile([1, 4], dt.float32, tag="warmin")
            warm_out = dram_pool.tile([1, 4], dt.float32, tag="warmout")
            warm_sb = v_pool.tile([1, 4], dt.float32, tag="warmsb")
            nc.vector.memset(warm_sb[:], 0.0)
            nc.gpsimd.dma_start(out=warm_in[:], in_=warm_sb[:])
            nc.gpsimd.collective_compute(
                "AllReduce", ALU.add, replica_groups=groups,
                ins=[warm_in.opt()], outs=[warm_out.opt()],
            )

            # batched input loads; issue spread over 4 engine queues so
            # descriptor generation isn't serialized on one sequencer
            qs = [nc.sync, nc.scalar, nc.sync, nc.scalar]
            for c in range(16):
                qs[c % 4].dma_start(
                    out=wsb[:].rearrange("p (t f) -> p t f", t=NT)[:, 2 * c:2 * c + 2, :],
                    in_=wmov_d[:].transpose([1, 0, 2])[:, 2 * c:2 * c + 2, :],
                )
            for c in range(2):
                qs[c].dma_start(
                    out=xblk[:].rearrange("p (t f) -> p t f", t=NT)[:, 16 * c:16 * c + 16, :],
                    in_=xblk_d[:].transpose([1, 0, 2])[:, 16 * c:16 * c + 16, :],
                )
            qs[2].dma_start(
                out=xall[:].rearrange("p (t f) -> p t f", t=NT),
                in_=xall_d[:].transpose([1, 0, 2]),
            )
            qs[3].dma_start(out=onesblk[:], in_=ones_d[:])

            # persistent logits tiles, one [128(4n,32b), 2*K] per W-tile
            ltiles = [lpool.tile([128, 2 * K], dt.float32, tag="L", name=f"L{t}")
                      for t in range(NT)]

            def allreduce_squash(s_ps0, s_ps1, last, it):
                """PSUM s halves -> AllReduce -> squash -> fresh vrep tile."""
                sfull = v_pool.tile([B, KO], dt.float32, tag="sfull",
                                    name=f"sfull{it}", bufs=2)
                vf32 = v_pool.tile([B, KO], dt.float32, tag="vf32",
                                   name=f"vf32{it}", bufs=2)
                sq1 = v_pool.tile([B, KO], dt.float32, tag="sq1",
                                  name=f"sq1{it}", bufs=2)
                nrm = v_pool.tile([B, K], dt.float32, tag="nrm",
                                  name=f"nrm{it}", bufs=3)
                nrm1 = v_pool.tile([B, K], dt.float32, tag="nrm1",
                                   name=f"nrm1{it}", bufs=3)
                den = v_pool.tile([B, K], dt.float32, tag="den",
                                  name=f"den{it}", bufs=3)
                rden = v_pool.tile([B, K], dt.float32, tag="rden",
                                   name=f"rden{it}", bufs=3)
                scal = v_pool.tile([B, K], dt.float32, tag="scal",
                                   name=f"scal{it}", bufs=3)
                nc.scalar.copy(out=sfull[:, 0:512], in_=s_ps0[:])
                nc.scalar.copy(out=sfull[:, 512:1024], in_=s_ps1[:])
                cc_in = dram_pool.tile([B, KO], dt.float32, tag="ccin")
                cc_out = dram_pool.tile([B, KO], dt.float32, tag="ccout")
                nc.gpsimd.dma_start(out=cc_in[:], in_=sfull[:])
                nc.gpsimd.collective_compute(
                    "AllReduce", ALU.add, replica_groups=groups,
                    ins=[cc_in.opt()], outs=[cc_out.opt()],
                )
                sred = v_pool.tile([B, KO], dt.float32, tag="sred",
                                   name=f"sred{it}", bufs=2)
                nc.gpsimd.dma_start(out=sred[:], in_=cc_out[:])
                # squash: v = s * nrm/((1+nrm)*sqrt(nrm)), nrm = sum_o s^2
                nc.scalar.activation(out=sq1[:], in_=sred[:], func=AF.Square)
                nc.vector.reduce_sum(
                    out=nrm[:], in_=sq1[:].rearrange("p (k o) -> p k o", k=K),
                    axis=mybir.AxisListType.X,
                )
                nc.vector.tensor_scalar_add(nrm1[:], nrm[:], 1.0)
                nc.scalar.activation(out=den[:], in_=nrm[:], func=AF.Sqrt)
                nc.vector.tensor_mul(den[:], den[:], nrm1[:])
                nc.vector.reciprocal(rden[:], den[:])
                nc.vector.tensor_mul(scal[:], nrm[:], rden[:])
                nc.vector.tensor_mul(
                    vf32[:].rearrange("p (k o) -> p k o", k=K),
                    sred[:].rearrange("p (k o) -> p k o", k=K),
                    scal[:].unsqueeze(2).broadcast_to((B, K, OC)),
                )
                if last:
                    nc.gpsimd.dma_start(out=vout_d[:], in_=vf32[:])
                    return None
                vbf = v_pool.tile([B, KO], dt.bfloat16, tag="vbf",
                                  name=f"vbf{it}", bufs=2)
                vrep = v_pool.tile([128, KO], dt.bfloat16, tag="vrep",
                                   name=f"vrep{it}", bufs=2)
                nc.vector.tensor_copy(vbf[:], vf32[:])
                for r in range(4):
                    nc.gpsimd.dma_start(
                        out=vrep[r * 32:(r + 1) * 32, :], in_=vbf[:]
                    )
                return vrep

            # ---------- pass A: s0 = (1/K) sum_n priors (direct matmul) -----
            s0a = spsum_pool.tile([B, 512], dt.float32, tag="sacc0")
            s0b = spsum_pool.tile([B, 512], dt.float32, tag="sacc1")
            for t in range(NT):
                nc.tensor.matmul(
                    out=s0a[:], lhsT=xall[:, t * B:(t + 1) * B],
                    rhs=wsb[:, t * KO:t * KO + 512],
                    start=(t == 0), stop=(t == NT - 1),
                )
                nc.tensor.matmul(
                    out=s0b[:], lhsT=xall[:, t * B:(t + 1) * B],
                    rhs=wsb[:, t * KO + 512:(t + 1) * KO],
                    start=(t == 0), stop=(t == NT - 1),
                )
            vrep = allreduce_squash(s0a, s0b, last=False, it=0)

            # ---------- passes B (iter1) and C (iter2) ----------------------
            for it in (1, 2):
                sa = spsum_pool.tile([B, 512], dt.float32, tag="sacc0")
                sb = spsum_pool.tile([B, 512], dt.float32, tag="sacc1")
                for t in range(NT):
                    pp = [ppsum_pool.tile([128, KO], dt.float32, tag="pp",
                                          name=f"pp{it}_{t}_{s}")
                          for s in (0, 1)]
                    for s in (0, 1):
                        lhs = xblk[s * 64:(s + 1) * 64, t * 128:(t + 1) * 128]
                        for h in (0, 1):
                            nc.tensor.matmul(
                                out=pp[s][:, h * 512:(h + 1) * 512], lhsT=lhs,
                                rhs=wsb[s * 64:(s + 1) * 64,
                                        t * KO + h * 512:t * KO + (h + 1) * 512],
                                start=True, stop=True, skip_group_check=True,
                            )
                    psb = psb_pool.tile([128, 2 * KO], dt.bfloat16, tag="psb",
                                        name=f"psb{it}_{t}")
                    for s in (0, 1):
                        nc.scalar.copy(out=psb[:, s * KO:(s + 1) * KO],
                                       in_=pp[s][:])
                    # t = P * v_rep over both groups at once
                    tt = t_pool.tile([128, 2 * KO], dt.bfloat16, tag="t",
                                     name=f"t{it}_{t}")
                    nc.vector.tensor_mul(
                        tt[:].rearrange("p (g f) -> p g f", g=2),
                        psb[:].rearrange("p (g f) -> p g f", g=2),
                        vrep[:].unsqueeze(1).broadcast_to((128, 2, KO)),
                    )
                    # tree reduce over o (bf16 2x adds, last level strided)
                    t3 = tt[:].rearrange("p (gk o) -> p gk o", o=32)
                    u1 = u_pool.tile([128, 64 * 16], dt.bfloat16, tag="u1",
                                     name=f"u1_{it}_{t}")
                    u1v = u1[:].rearrange("p (gk o) -> p gk o", o=16)
                    nc.vector.tensor_add(u1v, t3[:, :, 0:16], t3[:, :, 16:32])
                    u2 = u_pool.tile([128, 64 * 8], dt.bfloat16, tag="u2",
                                     name=f"u2_{it}_{t}")
                    u2v = u2[:].rearrange("p (gk o) -> p gk o", o=8)
                    nc.vector.tensor_add(u2v, u1v[:, :, 0:8], u1v[:, :, 8:16])
                    u3 = u_pool.tile([128, 64 * 4], dt.bfloat16, tag="u3",
                                     name=f"u3_{it}_{t}")
                    u3v = u3[:].rearrange("p (gk o) -> p gk o", o=4)
                    nc.vector.tensor_add(u3v, u2v[:, :, 0:4], u2v[:, :, 4:8])
                    u4 = u_pool.tile([128, 64 * 2], dt.bfloat16, tag="u4",
                                     name=f"u4_{it}_{t}")
                    u4v = u4[:].rearrange("p (gk o) -> p gk o", o=2)
                    nc.vector.tensor_add(u4v, u3v[:, :, 0:2], u3v[:, :, 2:4])
                    if it == 1:
                        nc.vector.tensor_add(
                            ltiles[t][:].unsqueeze(2),
                            u4v[:, :, 0:1], u4v[:, :, 1:2])
                    else:
                        dtmp = sm_pool.tile([128, 2 * K], dt.float32, tag="dtmp",
                                            name=f"dtmp{it}_{t}")
                        nc.vector.tensor_add(
                            dtmp[:].unsqueeze(2),
                            u4v[:, :, 0:1], u4v[:, :, 1:2])
                        nc.vector.tensor_add(ltiles[t][:], ltiles[t][:], dtmp[:])
                    # fused exp + o-expand per group, with Z accumulator
                    eexp = e_pool.tile([128, 2 * KO], dt.bfloat16, tag="eexp",
                                       name=f"eexp{it}_{t}")
                    zacc = sm_pool.tile([128, 2], dt.float32, tag="zacc",
                                        name=f"zacc{it}_{t}")
                    for g in (0, 1):
                        nc.scalar.activation(
                            out=eexp[:, g * KO:(g + 1) * KO].rearrange(
                                "p (k o) -> p k o", k=K),
                            in_=ltiles[t][:, g * K:(g + 1) * K]
                                .unsqueeze(2).broadcast_to((128, K, OC)),
                            func=AF.Exp,
                            accum_out=zacc[:, g:g + 1],
                        )
                    zr = sm_pool.tile([128, 2], dt.float32, tag="zr",
                                      name=f"zr{it}_{t}")
                    nc.vector.reciprocal(zr[:], zacc[:])
                    # wp = eexp * P  (unnormalized); 1/Z folded into zblk
                    wp = wp_pool.tile([128, 2 * KO], dt.bfloat16, tag="wp",
                                      name=f"wp{it}_{t}")
                    nc.vector.tensor_mul(wp[:], eexp[:], psb[:])
                    for g in (0, 1):
                        zblk = sm_pool.tile([128, B], dt.bfloat16, tag="zblk",
                                            name=f"zblk{it}_{t}_{g}")
                        # onesblk carries OC(=32)x so zblk = 32*delta/Zacc
                        nc.vector.tensor_scalar_mul(
                            zblk[:], onesblk[:], zr[:, g:g + 1])
                        gg = 2 * t + g
                        nc.tensor.matmul(
                            out=sa[:], lhsT=zblk[:],
                            rhs=wp[:, g * KO:g * KO + 512],
                            start=(gg == 0), stop=(gg == NGRP - 1),
                            skip_group_check=True,
                        )
                        nc.tensor.matmul(
                            out=sb[:], lhsT=zblk[:],
                            rhs=wp[:, g * KO + 512:(g + 1) * KO],
                            start=(gg == 0), stop=(gg == NGRP - 1),
                            skip_group_check=True,
                        )
                vrep = allreduce_squash(sa, sb, last=(it == 2), it=it)

    nc.compile()
    return nc


def _prep_inputs(x, route_weights):
    """Host-side shard + layout prep. Returns per-core in_maps."""
    bf16 = ml_dtypes.bfloat16
    xw = x.astype(np.float32)
    W = route_weights.astype(np.float32)
    in_maps = []
    for c in range(NCORES):
        n0 = c * NLOC
        xc = xw[:, n0:n0 + NLOC, :]          # [B, 256, IC]
        Wc = W[n0:n0 + NLOC]                 # [256, K, IC, OC]
        # wmov[t][s*64 + j*16 + i, k*OC + o] = W[8t+4s+j, k, i, o]
        wm = Wc.reshape(NT, 8, K, IC, OC)          # [t, node, k, i, o]
        wm = wm.transpose(0, 1, 3, 2, 4)           # [t, node, i, k, o]
        wmov = np.ascontiguousarray(
            wm.reshape(NT, 128, KO)).astype(bf16)
        # xblk[t][s*64 + j*16 + i, j'*32 + b] = x[b, 8t+4s+j, i] * (j==j')
        xb = np.zeros((NT, 2, 4, IC, 4, B), np.float32)
        xg = xc.transpose(1, 2, 0).reshape(NT, 2, 4, IC, B)  # [t,s,j,i,b]
        for j in range(4):
            xb[:, :, j, :, j, :] = xg[:, :, j, :, :]
        xblk = np.ascontiguousarray(
            xb.reshape(NT, 128, 128)).astype(bf16)
        # xall[t][(s*4+j)*16 + i, b] = x[b, n, i] / K
        xall = np.ascontiguousarray(
            (xg / K).reshape(NT, 128, B)).astype(bf16)
        # ones: delta(b,b') scaled by OC=32 to cancel the o-expansion in
        # the exp accumulator (Zacc = 32 * sum_k exp)
        ones = np.zeros((128, B), np.float32)
        for j in range(4):
            ones[j * 32 + np.arange(32), np.arange(32)] = float(OC)
        onesblk = ones.astype(bf16)
        in_maps.append({
            "wmov": wmov, "xblk": xblk, "xall": xall, "onesblk": onesblk,
        })
    return in_maps


def _get_nc():
    if "nc" not in _CACHE:
        _CACHE["nc"] = _build_bass()
    return _CACHE["nc"]


def kernel(x, route_weights, _trace=False, _trace_kwargs=None):
    from concourse.bass_utils import run_bass_kernel_spmd

    nc = _get_nc()
    in_maps = _prep_inputs(np.asarray(x), np.asarray(route_weights))
    res = run_bass_kernel_spmd(
        nc, in_maps, core_ids=list(range(NCORES)),
        trace=_trace, **(_trace_kwargs or {}),
    )
    out = res.results[0]["vout"].astype(np.float32)       # [B, K*OC]
    full = out.reshape(B, 1, K, 1, OC)
    if _trace:
        return full, res
    return full


# revision 25
# speedup vs baseline: 1.0418x; 1.0108x over previous
"""CapsuleLayer routing kernel for 8 Trainium2 NeuronCores.

Problem (full shapes): x [B=32, N=2048, IC=16] fp32,
route_weights [N=2048, K=32, IC=16, OC=32] fp32.
  priors = einsum('bni,nkio->bnko', x, W)
  3 routing iterations (softmax over K, weighted sum over N, squash)
  output = squash(s2) shaped [B, 1, K, 1, OC].

Sharding: N (nodes) is sharded 8 ways (256 nodes/core); the
route_weights shard (bf16, 8.4MB) stays SBUF-resident; priors are
recomputed on the PE each routing pass and never touch HBM.  The only
cross-core traffic is an AllReduce of s [B, K*OC] (128KB fp32) per
iteration.

Per-core per-pass structure (32 W-tiles of 8 nodes = 2 groups of 4):
  PE   : priors P[(4n,32b),(k,o)] via block-diag-x stationary matmuls
  ACT  : P PSUM->SBUF bf16 copy; fused exp(L)+o-expand with accumulated Z
  DVE  : t = P*v_rep; d = tree-reduce_o(t); zblk = ones*(1/Z);
         wp = exp_expanded * P
  PE   : s += zblk.T @ wp  (sums the 4 nodes and applies the softmax
         1/Z; PSUM-accumulates across all groups)
"""

import numpy as np
import ml_dtypes

B, NLOC, K, IC, OC = 32, 256, 32, 16, 32
NCORES = 8
N = NLOC * NCORES
KO = K * OC            # 1024
NT = NLOC // 8         # 32 W-tiles of 8 nodes
NGRP = NLOC // 4       # 64 groups of 4 nodes

_CACHE = {}


def _build_bass():
    import concourse.bass as bass
    import concourse.mybir as mybir
    from concourse import bacc, tile

    dt = mybir.dt
    AF = mybir.ActivationFunctionType
    ALU = mybir.AluOpType

    nc = bacc.Bacc("TRN2", target_bir_lowering=False)

    wmov_d = nc.declare_dram_parameter("wmov", [NT, 128, KO], dt.bfloat16, isOutput=False)
    xblk_d = nc.declare_dram_parameter("xblk", [NT, 128, 128], dt.bfloat16, isOutput=False)
    xall_d = nc.declare_dram_parameter("xall", [NT, 128, B], dt.bfloat16, isOutput=False)
    ones_d = nc.declare_dram_parameter("onesblk", [128, B], dt.bfloat16, isOutput=False)
    vout_d = nc.declare_dram_parameter("vout", [B, KO], dt.float32, isOutput=True)

    groups = [list(range(NCORES))]

    with tile.TileContext(nc) as tc:
        with (
            tc.tile_pool(name="wsb", bufs=1) as wpool,
            tc.tile_pool(name="persist", bufs=1) as ppool,
            tc.tile_pool(name="ltiles", bufs=NT) as lpool,
            tc.tile_pool(name="psb", bufs=4) as psb_pool,
            tc.tile_pool(name="tsb", bufs=4) as t_pool,
            tc.tile_pool(name="tree", bufs=2) as u_pool,
            tc.tile_pool(name="wp", bufs=4) as wp_pool,
            tc.tile_pool(name="eexp", bufs=4) as e_pool,
            tc.tile_pool(name="sm", bufs=6) as sm_pool,
            tc.tile_pool(name="vv", bufs=1) as v_pool,
            tc.tile_pool(name="ppsum", bufs=3, space="PSUM") as ppsum_pool,
            tc.tile_pool(name="spsum", bufs=1, space="PSUM") as spsum_pool,
            tc.tile_pool(name="dram", bufs=4, space="DRAM") as dram_pool,
        ):
            wsb = wpool.tile([128, NT * KO], dt.bfloat16, tag="wsb")
            xblk = ppool.tile([128, NT * 128], dt.bfloat16, tag="xblk")
            xall = ppool.tile([128, NT * B], dt.bfloat16, tag="xall")
            onesblk = ppool.tile([128, B], dt.bfloat16, tag="ones")

            # CC warmup: a tiny AllReduce issued first, overlapping the
            # input loads, absorbs the collective stack's cold start.
            warm_in = dram_pool.tile([1, 4], dt.float32, tag="warmin")
            warm_out = dram_pool.tile([1, 4], dt.float32, tag="warmout")
            warm_sb = v_pool.tile([1, 4], dt.float32, tag="warmsb")
            nc.vector.memset(warm_sb[:], 0.0)
            nc.gpsimd.dma_start(out=warm_in[:], in_=warm_sb[:])
            nc.gpsimd.collective_compute(
                "AllReduce", ALU.add, replica_groups=groups,
                ins=[warm_in.opt()], outs=[warm_out.opt()],
            )

            # batched input loads (big DMAs, several queues)
            for c in range(8):
                nc.sync.dma_start(
                    out=wsb[:].rearrange("p (t f) -> p t f", t=NT)[:, 4 * c:4 * c + 4, :],
                    in_=wmov_d[:].transpose([1, 0, 2])[:, 4 * c:4 * c + 4, :],
                )
            for c in range(2):
                nc.sync.dma_start(
                    out=xblk[:].rearrange("p (t f) -> p t f", t=NT)[:, 16 * c:16 * c + 16, :],
                    in_=xblk_d[:].transpose([1, 0, 2])[:, 16 * c:16 * c + 16, :],
                )
            nc.sync.dma_start(
                out=xall[:].rearrange("p (t f) -> p t f", t=NT),
                in_=xall_d[:].transpose([1, 0, 2]),
            )
            nc.sync.dma_start(out=onesblk[:], in_=ones_d[:])

            # persistent logits tiles, one [128(4n,32b), 2*K] per W-tile
            ltiles = [lpool.tile([128, 2 * K], dt.float32, tag="L", name=f"L{t}")
                      for t in range(NT)]

            def allreduce_squash(s_ps0, s_ps1, last, it):
                """PSUM s halves -> AllReduce -> squash -> fresh vrep tile."""
                sfull = v_pool.tile([B, KO], dt.float32, tag="sfull",
                                    name=f"sfull{it}", bufs=2)
                vf32 = v_pool.tile([B, KO], dt.float32, tag="vf32",
                                   name=f"vf32{it}", bufs=2)
                sq1 = v_pool.tile([B, KO], dt.float32, tag="sq1",
                                  name=f"sq1{it}", bufs=2)
                nrm = v_pool.tile([B, K], dt.float32, tag="nrm",
                                  name=f"nrm{it}", bufs=3)
                nrm1 = v_pool.tile([B, K], dt.float32, tag="nrm1",
                                   name=f"nrm1{it}", bufs=3)
                den = v_pool.tile([B, K], dt.float32, tag="den",
                                  name=f"den{it}", bufs=3)
                rden = v_pool.tile([B, K], dt.float32, tag="rden",
                                   name=f"rden{it}", bufs=3)
                scal = v_pool.tile([B, K], dt.float32, tag="scal",
                                   name=f"scal{it}", bufs=3)
                nc.scalar.copy(out=sfull[:, 0:512], in_=s_ps0[:])
                nc.scalar.copy(out=sfull[:, 512:1024], in_=s_ps1[:])
                cc_in = dram_pool.tile([B, KO], dt.float32, tag="ccin")
                cc_out = dram_pool.tile([B, KO], dt.float32, tag="ccout")
                nc.gpsimd.dma_start(out=cc_in[:], in_=sfull[:])
                nc.gpsimd.collective_compute(
                    "AllReduce", ALU.add, replica_groups=groups,
                    ins=[cc_in.opt()], outs=[cc_out.opt()],
                )
                sred = v_pool.tile([B, KO], dt.float32, tag="sred",
                                   name=f"sred{it}", bufs=2)
                nc.gpsimd.dma_start(out=sred[:], in_=cc_out[:])
                # squash: v = s * nrm/((1+nrm)*sqrt(nrm)), nrm = sum_o s^2
                nc.scalar.activation(out=sq1[:], in_=sred[:], func=AF.Square)
                nc.vector.reduce_sum(
                    out=nrm[:], in_=sq1[:].rearrange("p (k o) -> p k o", k=K),
                    axis=mybir.AxisListType.X,
                )
                nc.vector.tensor_scalar_add(nrm1[:], nrm[:], 1.0)
                nc.scalar.activation(out=den[:], in_=nrm[:], func=AF.Sqrt)
                nc.vector.reciprocal(rden[:], nrm1[:])
                nc.vector.tensor_mul(scal[:], den[:], rden[:])
                nc.vector.tensor_mul(
                    vf32[:].rearrange("p (k o) -> p k o", k=K),
                    sred[:].rearrange("p (k o) -> p k o", k=K),
                    scal[:].unsqueeze(2).broadcast_to((B, K, OC)),
                )
                if last:
                    nc.gpsimd.dma_start(out=vout_d[:], in_=vf32[:])
                    return None
                vbf = v_pool.tile([B, KO], dt.bfloat16, tag="vbf",
                                  name=f"vbf{it}", bufs=2)
                vrep = v_pool.tile([128, KO], dt.bfloat16, tag="vrep",
                                   name=f"vrep{it}", bufs=2)
                nc.vector.tensor_copy(vbf[:], vf32[:])
                for r in range(4):
                    nc.gpsimd.dma_start(
                        out=vrep[r * 32:(r + 1) * 32, :], in_=vbf[:]
                    )
                return vrep

            # ---------- pass A: s0 = (1/K) sum_n priors (direct matmul) -----
            s0a = spsum_pool.tile([B, 512], dt.float32, tag="sacc0")
            s0b = spsum_pool.tile([B, 512], dt.float32, tag="sacc1")
            for t in range(NT):
                nc.tensor.matmul(
                    out=s0a[:], lhsT=xall[:, t * B:(t + 1) * B],
                    rhs=wsb[:, t * KO:t * KO + 512],
                    start=(t == 0), stop=(t == NT - 1),
                )
                nc.tensor.matmul(
                    out=s0b[:], lhsT=xall[:, t * B:(t + 1) * B],
                    rhs=wsb[:, t * KO + 512:(t + 1) * KO],
                    start=(t == 0), stop=(t == NT - 1),
                )
            vrep = allreduce_squash(s0a, s0b, last=False, it=0)

            # ---------- passes B (iter1) and C (iter2) ----------------------
            for it in (1, 2):
                sa = spsum_pool.tile([B, 512], dt.float32, tag="sacc0")
                sb = spsum_pool.tile([B, 512], dt.float32, tag="sacc1")
                for t in range(NT):
                    pp = [ppsum_pool.tile([128, KO], dt.float32, tag="pp",
                                          name=f"pp{it}_{t}_{s}")
                          for s in (0, 1)]
                    for s in (0, 1):
                        lhs = xblk[s * 64:(s + 1) * 64, t * 128:(t + 1) * 128]
                        for h in (0, 1):
                            nc.tensor.matmul(
                                out=pp[s][:, h * 512:(h + 1) * 512], lhsT=lhs,
                                rhs=wsb[s * 64:(s + 1) * 64,
                                        t * KO + h * 512:t * KO + (h + 1) * 512],
                                start=True, stop=True, skip_group_check=True,
                            )
                    psb = psb_pool.tile([128, 2 * KO], dt.bfloat16, tag="psb",
                                        name=f"psb{it}_{t}")
                    for s in (0, 1):
                        nc.scalar.copy(out=psb[:, s * KO:(s + 1) * KO],
                                       in_=pp[s][:])
                    # t = P * v_rep over both groups at once
                    tt = t_pool.tile([128, 2 * KO], dt.bfloat16, tag="t",
                                     name=f"t{it}_{t}")
                    nc.vector.tensor_mul(
                        tt[:].rearrange("p (g f) -> p g f", g=2),
                        psb[:].rearrange("p (g f) -> p g f", g=2),
                        vrep[:].unsqueeze(1).broadcast_to((128, 2, KO)),
                    )
                    # tree reduce over o (bf16 2x adds, last level strided)
                    t3 = tt[:].rearrange("p (gk o) -> p gk o", o=32)
                    u1 = u_pool.tile([128, 64 * 16], dt.bfloat16, tag="u1",
                                     name=f"u1_{it}_{t}")
                    u1v = u1[:].rearrange("p (gk o) -> p gk o", o=16)
                    nc.vector.tensor_add(u1v, t3[:, :, 0:16], t3[:, :, 16:32])
                    u2 = u_pool.tile([128, 64 * 8], dt.bfloat16, tag="u2",
                                     name=f"u2_{it}_{t}")
                    u2v = u2[:].rearrange("p (gk o) -> p gk o", o=8)
                    nc.vector.tensor_add(u2v, u1v[:, :, 0:8], u1v[:, :, 8:16])
                    u3 = u_pool.tile([128, 64 * 4], dt.bfloat16, tag="u3",
                                     name=f"u3_{it}_{t}")
                    u3v = u3[:].rearrange("p (gk o) -> p gk o", o=4)
                    nc.vector.tensor_add(u3v, u2v[:, :, 0:4], u2v[:, :, 4:8])
                    u4 = u_pool.tile([128, 64 * 2], dt.bfloat16, tag="u4",
                                     name=f"u4_{it}_{t}")
                    u4v = u4[:].rearrange("p (gk o) -> p gk o", o=2)
                    nc.vector.tensor_add(u4v, u3v[:, :, 0:2], u3v[:, :, 2:4])
                    if it == 1:
                        nc.vector.tensor_add(
                            ltiles[t][:].unsqueeze(2),
                            u4v[:, :, 0:1], u4v[:, :, 1:2])
                    else:
                        dtmp = sm_pool.tile([128, 2 * K], dt.float32, tag="dtmp",
                                            name=f"dtmp{it}_{t}")
                        nc.vector.tensor_add(
                            dtmp[:].unsqueeze(2),
                            u4v[:, :, 0:1], u4v[:, :, 1:2])
                        nc.vector.tensor_add(ltiles[t][:], ltiles[t][:], dtmp[:])
                    # fused exp + o-expand per group, with Z accumulator
                    eexp = e_pool.tile([128, 2 * KO], dt.bfloat16, tag="eexp",
                                       name=f"eexp{it}_{t}")
                    zacc = sm_pool.tile([128, 2], dt.float32, tag="zacc",
                                        name=f"zacc{it}_{t}")
                    for g in (0, 1):
                        nc.scalar.activation(
                            out=eexp[:, g * KO:(g + 1) * KO].rearrange(
                                "p (k o) -> p k o", k=K),
                            in_=ltiles[t][:, g * K:(g + 1) * K]
                                .unsqueeze(2).broadcast_to((128, K, OC)),
                            func=AF.Exp,
                            accum_out=zacc[:, g:g + 1],
                        )
                    zr = sm_pool.tile([128, 2], dt.float32, tag="zr",
                                      name=f"zr{it}_{t}")
                    nc.vector.reciprocal(zr[:], zacc[:])
                    # wp = eexp * P  (unnormalized); 1/Z folded into zblk
                    wp = wp_pool.tile([128, 2 * KO], dt.bfloat16, tag="wp",
                                      name=f"wp{it}_{t}")
                    nc.vector.tensor_mul(wp[:], eexp[:], psb[:])
                    for g in (0, 1):
                        zblk = sm_pool.tile([128, B], dt.bfloat16, tag="zblk",
                                            name=f"zblk{it}_{t}_{g}")
                        # onesblk carries OC(=32)x so zblk = 32*delta/Zacc
                        nc.vector.tensor_scalar_mul(
                            zblk[:], onesblk[:], zr[:, g:g + 1])
                        gg = 2 * t + g
                        nc.tensor.matmul(
                            out=sa[:], lhsT=zblk[:],
                            rhs=wp[:, g * KO:g * KO + 512],
                            start=(gg == 0), stop=(gg == NGRP - 1),
                            skip_group_check=True,
                        )
                        nc.tensor.matmul(
                            out=sb[:], lhsT=zblk[:],
                            rhs=wp[:, g * KO + 512:(g + 1) * KO],
                            start=(gg == 0), stop=(gg == NGRP - 1),
                            skip_group_check=True,
                        )
                vrep = allreduce_squash(sa, sb, last=(it == 2), it=it)

    nc.compile()
    return nc


def _prep_inputs(x, route_weights):
    """Host-side shard + layout prep. Returns per-core in_maps."""
    bf16 = ml_dtypes.bfloat16
    xw = x.astype(np.float32)
    W = route_weights.astype(np.float32)
    in_maps = []
    for c in range(NCORES):
        n0 = c * NLOC
        xc = xw[:, n0:n0 + NLOC, :]          # [B, 256, IC]
        Wc = W[n0:n0 + NLOC]                 # [256, K, IC, OC]
        # wmov[t][s*64 + j*16 + i, k*OC + o] = W[8t+4s+j, k, i, o]
        wm = Wc.reshape(NT, 8, K, IC, OC)          # [t, node, k, i, o]
        wm = wm.transpose(0, 1, 3, 2, 4)           # [t, node, i, k, o]
        wmov = np.ascontiguousarray(
            wm.reshape(NT, 128, KO)).astype(bf16)
        # xblk[t][s*64 + j*16 + i, j'*32 + b] = x[b, 8t+4s+j, i] * (j==j')
        xb = np.zeros((NT, 2, 4, IC, 4, B), np.float32)
        xg = xc.transpose(1, 2, 0).reshape(NT, 2, 4, IC, B)  # [t,s,j,i,b]
        for j in range(4):
            xb[:, :, j, :, j, :] = xg[:, :, j, :, :]
        xblk = np.ascontiguousarray(
            xb.reshape(NT, 128, 128)).astype(bf16)
        # xall[t][(s*4+j)*16 + i, b] = x[b, n, i] / K
        xall = np.ascontiguousarray(
            (xg / K).reshape(NT, 128, B)).astype(bf16)
        # ones: delta(b,b') scaled by OC=32 to cancel the o-expansion in
        # the exp accumulator (Zacc = 32 * sum_k exp)
        ones = np.zeros((128, B), np.float32)
        for j in range(4):
            ones[j * 32 + np.arange(32), np.arange(32)] = float(OC)
        onesblk = ones.astype(bf16)
        in_maps.append({
            "wmov": wmov, "xblk": xblk, "xall": xall, "onesblk": onesblk,
        })
    return in_maps


def _get_nc():
    if "nc" not in _CACHE:
        _CACHE["nc"] = _build_bass()
    return _CACHE["nc"]


def kernel(x, route_weights, _trace=False, _trace_kwargs=None):
    from concourse.bass_utils import run_bass_kernel_spmd

    nc = _get_nc()
    in_maps = _prep_inputs(np.asarray(x), np.asarray(route_weights))
    res = run_bass_kernel_spmd(
        nc, in_maps, core_ids=list(range(NCORES)),
        trace=_trace, **(_trace_kwargs or {}),
    )
    out = res.results[0]["vout"].astype(np.float32)       # [B, K*OC]
    full = out.reshape(B, 1, K, 1, OC)
    if _trace:
        return full, res
    return full
